# revision 2
# baseline (speedup 1.0000x reference)
"""JointNet (RNN-T joint network) Bass kernel for 8 Trainium2 NeuronCores.

Math:  h = tanh(enc @ w1[:640] [:,None,:] + dec @ w1[640:] [None,:,:] + b1)
       out = h @ w2 + b2      over the (B, T, U) grid.

Sharding: data-parallel over T (sequence parallel). Each of the 8 cores gets a
T-slice of 32, so its enc slab flattens to exactly 128 rows = one partition
tile. dec and all weights are replicated. No collectives.

v2 (bf16): the vocab matmul dominates (64 m-tiles x 5 k x 1024 cols per core).
fp32r streams slower than bf16 on real HW, so all matmuls run in bf16
(fp32 PSUM accumulation; measured rel-err of the all-bf16 pipeline is ~3e-3
vs the 2e-2 gate). enc/dec are transposed on the host (no PE transposes, no
identity), w1/w2 are pre-cast to bf16 on the host.

Per-core schedule:
  1. DMA bf16 encT/decT/w1 (per-ko chunks so projections start early); w2/b2
     on the gpsimd queue in parallel.
  2. epb[h, bt] = w1enc.T @ encT + b1, dp[h, bu] = w1dec.T @ decT, bf16
     matmuls interleaved per m-tile so k=0 tiles land first.
  3. Grid work in half-chunks of 1024 cols (16 t x 64 u): DVE broadcast-add
     (fp32 in -> bf16 out), ACT tanh in-place, then 8 m-tiles of vocab
     matmul (lhsT = ht k-tile, rhs = w2, fp32 PSUM).
  4. +b2 fused into PSUM->SBUF copyback (alternating DVE / ACT+GPSIMD to
     balance engines); 512KB contiguous output DMAs.
"""

import numpy as np
import ml_dtypes
from contextlib import ExitStack

import concourse.bass as bass
from concourse.bacc import Bacc
import concourse.mybir as mybir
import concourse.tile as tile

B, T, U = 4, 256, 64
D, H, V = 640, 640, 1024
NCORES = 8
TSH = T // NCORES          # 32 T rows per core
BT = B * TSH               # 128 (b, t) rows per core
BU = B * U                 # 256 (b, u) rows
GRID = BT * U              # 8192 grid points per core
P = 128
KD = D // P                # 5 contraction tiles for the input dim
KH = H // P                # 5 contraction tiles for the hidden dim
THALF = 16                 # t rows per half-chunk
HALF = THALF * U           # 1024 grid cols per half-chunk
MHALF = HALF // P          # 8 m-tiles per half-chunk
F32 = mybir.dt.float32
BF16 = mybir.dt.bfloat16
NPBF = ml_dtypes.bfloat16


def _build():
    nc = Bacc()
    encT = nc.dram_tensor("encT", [D, BT], BF16, kind="ExternalInput")
    decT = nc.dram_tensor("decT", [D, BU], BF16, kind="ExternalInput")
    w1 = nc.dram_tensor("w1", [2 * D, H], BF16, kind="ExternalInput")
    b1 = nc.dram_tensor("b1", [H], F32, kind="ExternalInput")
    w2 = nc.dram_tensor("w2", [H, V], BF16, kind="ExternalInput")
    b2 = nc.dram_tensor("b2", [V], F32, kind="ExternalInput")
    out = nc.dram_tensor("out", [GRID, V], F32, kind="ExternalOutput")

    with tile.TileContext(nc) as tc, ExitStack() as ctx:
        const = ctx.enter_context(tc.tile_pool(name="const", bufs=1))
        ht_pool = ctx.enter_context(tc.tile_pool(name="ht", bufs=3))
        osb_pool = ctx.enter_context(tc.tile_pool(name="osb", bufs=3))
        psum = ctx.enter_context(tc.tile_pool(name="psum", bufs=4, space="PSUM"))
        psum_s = ctx.enter_context(tc.tile_pool(name="psum_s", bufs=2, space="PSUM"))

        # --- constants (per-ko DMAs so the first proj m-tile unblocks early)
        encT_sb = const.tile([P, KD, BT], BF16, tag="encT")
        w1_sb = const.tile([P, 2 * KD, H], BF16, tag="w1")
        decT_sb = const.tile([P, KD, BU], BF16, tag="decT")
        for ko in range(KD):
            nc.sync.dma_start(encT_sb[:, ko, :], encT[:][ko * P:(ko + 1) * P, :])
        for ko in range(2 * KD):
            nc.sync.dma_start(w1_sb[:, ko, :], w1[:][ko * P:(ko + 1) * P, :])
        for ko in range(KD):
            nc.sync.dma_start(decT_sb[:, ko, :], decT[:][ko * P:(ko + 1) * P, :])
        b1_sb = const.tile([P, KH], F32, tag="b1")
        nc.sync.dma_start(b1_sb[:], b1[:].rearrange("(ko p) -> p ko", p=P))
        w2_sb = const.tile([P, KH, V], BF16, tag="w2")
        for ko in range(KH):
            nc.gpsimd.dma_start(w2_sb[:, ko, :], w2[:][ko * P:(ko + 1) * P, :])
        b2_sb = const.tile([P, V], F32, tag="b2")
        nc.gpsimd.dma_start(b2_sb[:], b2[:][None, :].to_broadcast((P, V)))

        # --- projections: epb = w1enc.T @ encT + b1, dp = w1dec.T @ decT
        epb = const.tile([P, KH, BT], F32, tag="epb")
        dp = const.tile([P, KH, BU], F32, tag="dp")
        for m in range(KH):
            pt = psum_s.tile([P, BU], F32, tag="ps", name="ps")
            for kd in range(KD):
                nc.tensor.matmul(
                    pt[:, :BT],
                    lhsT=w1_sb[:, kd, m * P:(m + 1) * P],
                    rhs=encT_sb[:, kd, :],
                    start=(kd == 0), stop=(kd == KD - 1),
                )
            nc.vector.tensor_scalar_add(epb[:, m, :], pt[:, :BT], b1_sb[:, m:m + 1])
            pt2 = psum_s.tile([P, BU], F32, tag="ps", name="ps")
            for kd in range(KD):
                nc.tensor.matmul(
                    pt2,
                    lhsT=w1_sb[:, KD + kd, m * P:(m + 1) * P],
                    rhs=decT_sb[:, kd, :],
                    start=(kd == 0), stop=(kd == KD - 1),
                )
            nc.any.tensor_copy(dp[:, m, :], pt2)

        # --- main grid loop, one half-chunk (16t x 64u) at a time ----------
        for b in range(B):
            for hf in range(2):
                t0 = b * TSH + hf * THALF
                ht = ht_pool.tile([P, KH, HALF], BF16, tag="ht")
                for k in range(KH):
                    # ht[:, k, t*64+u] = epb[:, k, t0+t] + dp[:, k, b*64+u]
                    nc.vector.tensor_tensor(
                        ht[:, k, :].rearrange("p (t u) -> p t u", u=U),
                        epb[:, k, t0:t0 + THALF][:, :, None]
                            .to_broadcast((P, THALF, U)),
                        dp[:, k, b * U:(b + 1) * U][:, None, :]
                            .to_broadcast((P, THALF, U)),
                        mybir.AluOpType.add,
                    )
                    nc.scalar.activation(ht[:, k, :], ht[:, k, :],
                                         mybir.ActivationFunctionType.Tanh)

                for mi in range(MHALF):
                    osb = osb_pool.tile([P, V], F32, tag="osb")
                    pts = [psum.tile([P, 512], F32, tag="mm", name="mm")
                           for _ in range(2)]
                    # k-outer / nh-inner: each ht lhsT tile feeds both halves
                    for k in range(KH):
                        for nh in range(2):
                            nc.tensor.matmul(
                                pts[nh][:],
                                lhsT=ht[:, k, mi * P:(mi + 1) * P],
                                rhs=w2_sb[:, k, nh * 512:(nh + 1) * 512],
                                start=(k == 0), stop=(k == KH - 1),
                            )
                    for nh in range(2):
                        sl = slice(nh * 512, (nh + 1) * 512)
                        if mi % 2 == 0:
                            # DVE: copyback with fused +b2
                            nc.vector.tensor_tensor(osb[:, sl], pts[nh][:],
                                                    b2_sb[:, sl],
                                                    mybir.AluOpType.add)
                        else:
                            # ACT copies PSUM->SBUF; idle GPSIMD adds b2
                            nc.scalar.copy(osb[:, sl], pts[nh][:])
                            nc.gpsimd.tensor_tensor(osb[:, sl], osb[:, sl],
                                                    b2_sb[:, sl],
                                                    mybir.AluOpType.add)
                    row0 = ((b * 2 + hf) * MHALF + mi) * P
                    nc.sync.dma_start(out[:][row0:row0 + P, :], osb[:])

    return nc


_NC_CACHE = {}


def _get_nc(key="bf16"):
    if key not in _NC_CACHE:
        nc = _build()
        if not nc.is_finalized():
            nc.finalize()
        _NC_CACHE[key] = nc
    return _NC_CACHE[key]


def make_in_maps(enc_state, dec_state, w1, b1, w2, b2):
    enc_state = np.ascontiguousarray(enc_state, dtype=np.float32)
    decT = np.ascontiguousarray(
        np.asarray(dec_state, dtype=np.float32).reshape(BU, D).T
    ).astype(NPBF)
    w1_bf = np.ascontiguousarray(w1, dtype=np.float32).astype(NPBF)
    w2_bf = np.ascontiguousarray(w2, dtype=np.float32).astype(NPBF)
    b1_f = np.ascontiguousarray(b1, dtype=np.float32)
    b2_f = np.ascontiguousarray(b2, dtype=np.float32)
    in_maps = []
    for c in range(NCORES):
        slab = enc_state[:, c * TSH:(c + 1) * TSH, :].reshape(BT, D)
        in_maps.append({
            "encT": np.ascontiguousarray(slab.T).astype(NPBF),
            "decT": decT,
            "w1": w1_bf,
            "b1": b1_f,
            "w2": w2_bf,
            "b2": b2_f,
        })
    return in_maps


def kernel(enc_state, dec_state, w1, b1, w2, b2):
    from concourse.bass_utils import run_bass_kernel_spmd

    nc = _get_nc()
    in_maps = make_in_maps(enc_state, dec_state, w1, b1, w2, b2)
    res = run_bass_kernel_spmd(nc, in_maps, core_ids=list(range(NCORES)))
    shards = [res.results[c]["out"].reshape(B, TSH, U, V) for c in range(NCORES)]
    return np.concatenate(shards, axis=1)


if __name__ == "__main__":
    rng = np.random.default_rng(0)
    ins = {
        "enc_state": rng.standard_normal((B, T, D), dtype=np.float32),
        "dec_state": rng.standard_normal((B, U, D), dtype=np.float32),
        "w1": rng.standard_normal((2 * D, H), dtype=np.float32) / np.sqrt(2 * D),
        "b1": rng.standard_normal((H,), dtype=np.float32) * 0.02,
        "w2": rng.standard_normal((H, V), dtype=np.float32) / np.sqrt(H),
        "b2": rng.standard_normal((V,), dtype=np.float32) * 0.02,
    }
    out = kernel(**ins)
    print(out.shape, out.dtype)
